# revision 6
# baseline (speedup 1.0000x reference)
"""Trainium2 Bass kernel for nn_Drug_PNAConv (GNN message passing, PNAConv).

v4 strategy (evolution of v3c):
  - Nodes partitioned by destination across 8 cores; host bins edges by dest
    degree into fixed chunks (g nodes x degree d), slots J-MAJOR
    (slot = j*d + k) so segmented min/max reduces see unit-stride inner axes.
  - Host pre-gathers BOTH endpoint features as fp8e4m3 streams xj8/xi8
    [128, S]; p1 = Wxj.xj + Wxi.xi computed in ONE fp8 DoubleRow matmul
    (pair weights [Wxj|Wxi]); bond term stays a bf16 16-partition matmul.
  - h1 = relu(p1 + b1p) evacuated straight to fp8; p2 = W2.h1 (fp8);
    segmented SUM via DoubleRow k-pair matmuls ([W2|W2] x h1 pairs).
  - p2 PSUM evacuated once to bf16 (p16); min/max via DVE tensor_reduce and
    msq = p16*p16 on bf16 SBUF (DVE 2x/4x modes); segmented SUMSQ via a
    single IDENT x msq broadcast-output matmul.
  - Per-chunk s,q evacuated with 1/d scale (s16 = s/d, q16 = q/d); the whole
    std chain (m2 = s16^2, v = q16 - m2, relu, sqrt(+eps)) runs BATCHED per
    512-node block instead of per chunk.
  - Degree scalers folded into per-degree-class post weights
    (Wsm_d = d*Wc0 + Wc1 for the s16 = s/d convention); b_pre2 folded into
    per-class correction vectors; b_post2 folded into the W_lin bias;
    LayerNorm mean-centering folded into W_lin (P_c).
  - Output written FEATURE-MAJOR bf16 (residual added from the bf16
    feature-major x tile via one fused scalar_tensor_tensor); the host does
    the final transpose during unshard. No PE transposes, no f32 node-major
    x load.
"""

import os
import sys

for _p in ("/opt/trn_rl_repo", os.path.expanduser("~/.axon_site/_ro/trn_rl_repo")):
    if os.path.isdir(_p) and _p not in sys.path:
        sys.path.insert(0, _p)

import numpy as np

import concourse.bass as bass
import concourse.bacc as bacc
import concourse.mybir as mybir
import concourse.tile as tile
from concourse.bass_utils import run_bass_kernel_spmd

F32 = mybir.dt.float32
BF16 = mybir.dt.bfloat16
FP8 = mybir.dt.float8e4
AF = mybir.ActivationFunctionType
OP = mybir.AluOpType
AX = mybir.AxisListType
DR = mybir.MatmulPerfMode.DoubleRow

N_CORES = 8
H = 128
T = 4
F_IN = 32
EC = 16
EPS = 1e-5
GROUP_COLS = 2048

# fp8 feature flags (fall back to bf16 paths when False)
FP8_PAIR = True   # host xj/xi streams fp8 + DoubleRow p1
FP8_H1 = True     # h1 evac to fp8, p2/segsum matmuls fp8 (segsum DoubleRow)

_DEG_HIST = np.array([0.0, 5000.0, 20000.0, 25000.0, 10000.0])
_BINS = np.arange(_DEG_HIST.size)
AVG_DEG_LOG = float((np.log(_BINS + 1.0) * _DEG_HIST).sum() / _DEG_HIST.sum())


def _ceil_to(x, m):
    return ((x + m - 1) // m) * m


# --------------------------------------------------------------------------
# Host-side planning (sharding + layout)
# --------------------------------------------------------------------------

class Plan:
    pass


def make_plan(src, dst, n_nodes, n_cores=N_CORES):
    assert n_nodes % n_cores == 0
    npc = n_nodes // n_cores
    p = Plan()
    p.n_nodes = n_nodes
    p.npc = npc
    p.n_cores = n_cores

    owner = dst // npc
    core_edges = []
    core_deg = []
    dmax = 0
    for c in range(n_cores):
        sel = np.nonzero(owner == c)[0]
        dloc = dst[sel] - c * npc
        deg = np.bincount(dloc, minlength=npc)
        dmax = max(dmax, int(deg.max()) if deg.size else 0)
        core_edges.append(sel)
        core_deg.append(deg)
    assert dmax <= 512, f"degree {dmax} too large"
    p.dmax = dmax

    n_d_max = np.zeros(dmax + 1, dtype=np.int64)
    for c in range(n_cores):
        cnt = np.bincount(core_deg[c], minlength=dmax + 1)
        n_d_max = np.maximum(n_d_max, cnt)

    # layout order: d = 1, 2, ..., dmax, then 0 (degree-0 nodes need no
    # chunks, so the final post block has no late chunk dependency)
    p.dorder = [d for d in range(1, dmax + 1) if n_d_max[d] > 0] + [0]
    sec_off = {}
    off = 0
    for d in p.dorder:
        sec_off[d] = off
        off += int(n_d_max[d])
    n_used = off
    p.N_layout = _ceil_to(max(n_used, 512), 512)
    p.n_used = n_used
    p.sec_off = sec_off

    # chunks in emission order: d1 first, then small sections (d >= 6), then
    # the bulk sections (2..5). Layout position is sec_off-based regardless.
    emit_ds = [d for d in p.dorder if d >= 1]
    emit_order = ([d for d in emit_ds if d == 1] +
                  [d for d in emit_ds if d >= 6] +
                  [d for d in emit_ds if 2 <= d <= 5])
    chunks = []  # (d, g, cols, slot_base, node_base)
    sbase = 0
    for d in emit_order:
        rem = int(n_d_max[d])
        nbase = sec_off[d]
        gmax = 512 // d
        if d >= 2:
            gmax = min(gmax, 256)  # s/q halves of one PSUM bank
        while rem > 0:
            g = min(rem, gmax)
            cols = _ceil_to(g * d, 128)
            chunks.append((d, g, cols, sbase, nbase))
            sbase += cols
            nbase += g
            rem -= g
    p.chunks = chunks
    p.S = sbase if sbase > 0 else 128

    # sections (layout ranges) per degree class
    p.sections = []
    for d in p.dorder:
        if d >= 1 and n_d_max[d] > 0:
            p.sections.append((d, sec_off[d], sec_off[d] + int(n_d_max[d])))
    p.n0 = int(n_d_max[0])
    p.n0_off = sec_off[0]
    p.dclasses = [d for (d, a, b) in p.sections]

    # d>=2 layout range (contiguous) for the batched std chain
    lo = min((a for (d, a, b) in p.sections if d >= 2), default=0)
    hi = max((b for (d, a, b) in p.sections if d >= 2), default=0)
    p.std_range = (lo, hi)

    # gather groups of consecutive chunks, total cols <= GROUP_COLS
    groups = []
    cur = None
    for ci, (d, g, cols, sb, nb) in enumerate(chunks):
        if cur is None or cur[1] + cols > GROUP_COLS:
            cur = [sb, cols, [ci]]
            groups.append(cur)
        else:
            cur[1] += cols
            cur[2].append(ci)
    p.groups = [tuple(x) for x in groups]

    # per-core node layout + slot->edge map (j-major within chunks)
    p.layout_nodes = []
    p.core_edges_sorted = []
    for c in range(n_cores):
        deg = core_deg[c]
        lay = np.full(p.N_layout, -1, dtype=np.int64)
        for d in p.dorder:
            ids = np.nonzero(deg == d)[0]
            lay[sec_off[d]:sec_off[d] + ids.size] = ids
        p.layout_nodes.append(lay)

        sel = core_edges[c]
        dloc = dst[sel] - c * npc
        eorder = np.argsort(dloc, kind="stable")
        sel_sorted = sel[eorder]
        starts = np.zeros(npc + 1, dtype=np.int64)
        starts[1:] = np.cumsum(deg)

        slot_edge = np.full(p.S, -1, dtype=np.int64)
        for (d, g, cols, sb, nb) in chunks:
            nodes = lay[nb:nb + g]
            real = np.nonzero(nodes >= 0)[0]
            ed = np.full((g, d), -1, dtype=np.int64)
            if real.size:
                rn = nodes[real]
                em = starts[rn][:, None] + np.arange(d)[None, :]
                ed[real] = sel_sorted[em]
            # j-major: slot = j*d + k
            slot_edge[sb:sb + g * d] = ed.ravel()
        p.core_edges_sorted.append(slot_edge)

    # post weight-blob layout (bf16): W1X, W2P, WLINC, then per class
    p.widx = dict(W1X=0, W2P=1, WLINC=2)
    p.wd_idx = {}
    p.cv_idx = {0: 0}
    bi, ci = 3, 1
    for d in p.dclasses:
        p.wd_idx[d] = bi
        bi += 1 if d == 1 else 4
        p.cv_idx[d] = ci
        ci += 1
    p.n_wblocks = bi
    p.n_cvrow = 1 + ci
    return p


def make_core_inputs(p, c, atom_x, bond_x, src, dst, W):
    import ml_dtypes
    F8 = ml_dtypes.float8_e4m3fn
    BF = ml_dtypes.bfloat16
    npc = p.npc
    lay = p.layout_nodes[c]
    slot_edge = p.core_edges_sorted[c]
    S = p.S

    valid = slot_edge >= 0
    se = np.maximum(slot_edge, 0)
    sdt = F8 if FP8_PAIR else BF
    xj_id = np.where(valid, src[se], 0)
    xj = atom_x[xj_id]
    xj[~valid] = 0.0
    xj8 = np.ascontiguousarray(xj.T.astype(sdt))
    xi_id = np.where(valid, dst[se], 0)
    xi = atom_x[xi_id]
    xi[~valid] = 0.0
    xi8 = np.ascontiguousarray(xi.T.astype(sdt))

    bondT = np.zeros((S, EC), dtype=BF)
    bondT[valid] = bond_x[slot_edge[valid]].astype(BF)
    bondT = np.ascontiguousarray(bondT.T)

    gid = np.where(lay >= 0, c * npc + lay, 0)
    x_layT = np.ascontiguousarray(atom_x[gid].T.astype(BF))

    m = dict(xj8=xj8, xi8=xi8, bondT=bondT, x_layT=x_layT)
    m.update(W)
    return m


def make_weights(inp, p):
    """Host-side weight folding. Returns dict of shared DRAM inputs."""
    import ml_dtypes
    BF = ml_dtypes.bfloat16
    F8 = ml_dtypes.float8_e4m3fn
    W_pre1, b_pre1 = np.asarray(inp["W_pre1"], np.float64), np.asarray(inp["b_pre1"], np.float64)
    W_pre2, b_pre2 = np.asarray(inp["W_pre2"], np.float64), np.asarray(inp["b_pre2"], np.float64)
    W_post1, b_post1 = np.asarray(inp["W_post1"], np.float64), np.asarray(inp["b_post1"], np.float64)
    W_post2, b_post2 = np.asarray(inp["W_post2"], np.float64), np.asarray(inp["b_post2"], np.float64)
    W_lin, b_lin = np.asarray(inp["W_lin"], np.float64), np.asarray(inp["b_lin"], np.float64)
    W_bond, b_bond = np.asarray(inp["W_bond"], np.float64), np.asarray(inp["b_bond"], np.float64)
    W_ee, b_ee = np.asarray(inp["W_ee"], np.float64), np.asarray(inp["b_ee"], np.float64)
    assert np.allclose(np.asarray(inp["ln_w"]), 1.0) and \
        np.allclose(np.asarray(inp["ln_b"]), 0.0), "ln affine not identity"

    def blockdiag(mats):
        n = len(mats)
        r, co = mats[0].shape
        out = np.zeros((n * r, n * co), dtype=np.float64)
        for t in range(n):
            out[t * r:(t + 1) * r, t * co:(t + 1) * co] = mats[t]
        return out

    W_be = W_bond @ W_ee
    b_be = b_bond @ W_ee + b_ee
    Wxi = blockdiag([W_pre1[t][0:F_IN] for t in range(T)])
    Wxj = blockdiag([W_pre1[t][F_IN:2 * F_IN] for t in range(T)])
    Wea_cat = np.concatenate([W_pre1[t][2 * F_IN:3 * F_IN] for t in range(T)], axis=1)
    W_bea = W_be @ Wea_cat
    b1p = b_pre1.reshape(H) + b_be @ Wea_cat
    W2bd = blockdiag([W_pre2[t] for t in range(T)])
    b2 = b_pre2.reshape(H)
    W1X = blockdiag([W_post1[t][0:F_IN] for t in range(T)])
    W1 = [[blockdiag([W_post1[t][F_IN + r * 5 * F_IN + a * F_IN:
                                 F_IN + r * 5 * F_IN + (a + 1) * F_IN]
                      for t in range(T)]) for a in range(5)] for r in range(3)]
    W2p = blockdiag([W_post2[t] for t in range(T)])
    b3 = b_post2.reshape(H)
    bp1 = b_post1.reshape(H)

    # LN centering + b3/b_lin fold
    P_c = np.eye(H) - np.ones((H, H)) / H
    WLINc = W_lin @ P_c
    blinc2 = (b3 @ W_lin + b_lin) @ P_c

    # edge-phase pair weights (fp8 or bf16)
    sdt = F8 if FP8_PAIR else BF
    wpair = np.concatenate([Wxj, Wxi], axis=1).astype(sdt)       # [128, 256]
    hdt = F8 if FP8_H1 else BF
    w2pair = np.concatenate([W2bd, W2bd], axis=1).astype(hdt)    # [128, 256]
    ident16 = np.eye(H).astype(BF)

    # per-degree-class post weights + corrections (s16 = s/d convention)
    wblocks = [W1X, W2p, WLINc]
    cvecs = []

    def wcomb(d):
        dc = max(d, 1.0)
        logdeg = np.log(dc + 1.0)
        amp, att = logdeg / AVG_DEG_LOG, AVG_DEG_LOG / logdeg
        return [W1[0][a] + amp * W1[1][a] + att * W1[2][a] for a in range(5)]

    Wc0 = wcomb(0)
    cvecs.append(np.sqrt(EPS) * Wc0[4].sum(axis=0))
    for d in p.dclasses:
        Wc = wcomb(d)
        if d == 1:
            wblocks.append(Wc[0] + Wc[1] + Wc[2] + Wc[3])
            cvecs.append(b2 @ (Wc[0] + Wc[1] + Wc[2] + Wc[3])
                         + np.sqrt(EPS) * Wc[4].sum(axis=0))
        else:
            wblocks.extend([d * Wc[0] + Wc[1], Wc[2], Wc[3], Wc[4]])
            cvecs.append(b2 @ (d * Wc[0] + Wc[1] + Wc[2] + Wc[3]))

    assert len(wblocks) == p.n_wblocks
    wmm = np.concatenate([np.asarray(w) for w in wblocks], axis=1).astype(BF)
    cvrow = np.concatenate([np.ones(H)] + cvecs).astype(BF)[None, :]
    assert cvrow.shape[1] == p.n_cvrow * 128, (cvrow.shape, p.n_cvrow)
    baux = np.stack([b1p, bp1, blinc2], axis=1).astype(np.float32)  # [128,3]
    return dict(
        wpair8=np.ascontiguousarray(wpair),
        w2pair8=np.ascontiguousarray(w2pair),
        ident16=np.ascontiguousarray(ident16),
        wbea16=np.ascontiguousarray(W_bea.astype(BF)),
        wmm=np.ascontiguousarray(wmm),
        cvrow=np.ascontiguousarray(cvrow),
        baux=np.ascontiguousarray(baux),
    )


# --------------------------------------------------------------------------
# Bass kernel builder
# --------------------------------------------------------------------------

def build_nc(p, n_nodes, debug=False):
    nc = bacc.Bacc("TRN2", target_bir_lowering=False, debug=debug)
    S, NL = p.S, p.N_layout
    NB = NL // 512
    SDT = FP8 if FP8_PAIR else BF16
    HDT = FP8 if FP8_H1 else BF16

    xj8_d = nc.dram_tensor("xj8", [128, S], SDT, kind="ExternalInput")
    xi8_d = nc.dram_tensor("xi8", [128, S], SDT, kind="ExternalInput")
    bondT_d = nc.dram_tensor("bondT", [EC, S], BF16, kind="ExternalInput")
    xlt_d = nc.dram_tensor("x_layT", [128, NL], BF16, kind="ExternalInput")
    wpair_d = nc.dram_tensor("wpair8", [128, 256], SDT, kind="ExternalInput")
    w2pair_d = nc.dram_tensor("w2pair8", [128, 256], HDT, kind="ExternalInput")
    ident_d = nc.dram_tensor("ident16", [128, 128], BF16, kind="ExternalInput")
    wbea_d = nc.dram_tensor("wbea16", [EC, 128], BF16, kind="ExternalInput")
    wmm_d = nc.dram_tensor("wmm", [128, p.n_wblocks * 128], BF16, kind="ExternalInput")
    cvrow_d = nc.dram_tensor("cvrow", [1, p.n_cvrow * 128], BF16, kind="ExternalInput")
    baux_d = nc.dram_tensor("baux", [128, 3], F32, kind="ExternalInput")
    out_d = nc.dram_tensor("out", [128, NL], BF16, kind="ExternalOutput")

    # blocks -> pieces; piece = (d, a, b) with [a,b) within block
    sec_all = p.sections + [(0, p.n0_off, p.n0_off + p.n0)]
    block_pieces = [[] for _ in range(NB)]
    for (d, a, b) in sec_all:
        if b <= a:
            continue
        t0, t1 = a // 512, (b - 1) // 512
        for t in range(t0, t1 + 1):
            pa, pb_ = max(a, t * 512), min(b, (t + 1) * 512)
            if pb_ > pa:
                block_pieces[t].append((d, pa, pb_))

    # block -> last chunk index contributing to it
    last_chunk = [-1] * NB
    for ci, (d, g, cols, sb, nb) in enumerate(p.chunks):
        for t in range(nb // 512, min((nb + g - 1) // 512, NB - 1) + 1):
            last_chunk[t] = max(last_chunk[t], ci)
    for t in range(1, NB):
        last_chunk[t] = max(last_chunk[t], last_chunk[t - 1])

    slo, shi = p.std_range

    with tile.TileContext(nc) as tc:
        from contextlib import ExitStack
        with ExitStack() as ctx:
            # small edge-phase consts first: unblock the first chunks fast
            cpool = ctx.enter_context(tc.tile_pool(name="consts", bufs=1))
            wpair8 = cpool.tile([128, 256], SDT)
            nc.sync.dma_start(wpair8[:], wpair_d[:])
            w2pair8 = cpool.tile([128, 256], HDT)
            nc.sync.dma_start(w2pair8[:], w2pair_d[:])
            ident16 = cpool.tile([128, 128], BF16)
            nc.sync.dma_start(ident16[:], ident_d[:])
            wbea16 = cpool.tile([EC, 128], BF16)
            nc.sync.dma_start(wbea16[:], wbea_d[:])
            baux = cpool.tile([128, 3], F32)
            nc.sync.dma_start(baux[:], baux_d[:])

            gtiles = {}
            gp = ctx.enter_context(tc.tile_pool(name="edge_gath", bufs=3))

            def emit_gather(gi):
                gsb, gcols, _ = p.groups[gi]
                t = gp.tile([128, 2 * GROUP_COLS], SDT, tag="pair")
                gtiles[gi] = (t, gcols)
                nc.sync.dma_start(t[:, 0:gcols], xj8_d[:, gsb:gsb + gcols])
                nc.sync.dma_start(t[:, GROUP_COLS:GROUP_COLS + gcols],
                                  xi8_d[:, gsb:gsb + gcols])

            emit_gather(0)

            # post-phase consts (needed only once blocks start draining)
            wmm = cpool.tile([128, p.n_wblocks * 128], BF16)
            nc.sync.dma_start(wmm[:], wmm_d[:])
            cvrow = cpool.tile([1, p.n_cvrow * 128], BF16)
            nc.sync.dma_start(cvrow[:], cvrow_d[:])
            epsc = cpool.tile([128, 1], F32)
            nc.vector.memset(epsc[:], EPS)
            onescol = cpool.tile([128, 1], BF16)
            nc.vector.memset(onescol[:], 1.0)
            ones512 = cpool.tile([1, 512], BF16)
            nc.vector.memset(ones512[:], 1.0)

            def WB(i):
                return wmm[:, i * 128:(i + 1) * 128]

            def CV(di):
                i = p.cv_idx[di] + 1
                return cvrow[:, i * 128:(i + 1) * 128]

            ONESROW = cvrow[:, 0:128]
            B1P, BP1, BLINC2 = (baux[:, i:i + 1] for i in range(3))
            wpairv = wpair8[:].rearrange("p (two m) -> p two m", two=2)
            w2pairv = w2pair8[:].rearrange("p (two m) -> p two m", two=2)
            w2one = w2pair8[:, 0:128]

            # persistent aggregate arrays
            aggp = ctx.enter_context(tc.tile_pool(name="agg", bufs=1))
            s16_agg = aggp.tile([128, NL], BF16)
            q16_agg = aggp.tile([128, NL], BF16)
            mn16_agg = aggp.tile([128, NL], BF16)
            mx16_agg = aggp.tile([128, NL], BF16)
            std16_agg = aggp.tile([128, NL], BF16)

            ep = ctx.enter_context(tc.tile_pool(name="edge_sb", bufs=4))
            pb = ctx.enter_context(tc.tile_pool(name="post_sb", bufs=2))
            xp = ctx.enter_context(tc.tile_pool(name="xfm_sb", bufs=3))
            epp = ctx.enter_context(tc.tile_pool(name="edge_ps", bufs=3, space="PSUM"))
            sqp = ctx.enter_context(tc.tile_pool(name="sq_ps", bufs=2, space="PSUM"))
            opp = ctx.enter_context(tc.tile_pool(name="op_ps", bufs=1, space="PSUM"))
            mpp = ctx.enter_context(tc.tile_pool(name="misc_ps", bufs=2, space="PSUM"))

            def emit_head(gi, ci):
                """p1 pair matmul + bond matmul + relu evac."""
                gsb, gcols, _ = p.groups[gi]
                pair_t, _ = gtiles[gi]
                d, g, cols, sb, nb = p.chunks[ci]
                loc = sb - gsb
                gd = g * d

                bond_t = ep.tile([EC, 512], BF16, tag="bond")
                nc.sync.dma_start(bond_t[:, 0:gd], bondT_d[:, sb:sb + gd])

                p1 = epp.tile([128, 512], F32, tag="mm")
                if FP8_PAIR:
                    rhs = pair_t[:, 0:2 * GROUP_COLS].rearrange(
                        "p (two n) -> p two n", two=2)[:, :, loc:loc + gd]
                    nc.tensor.matmul(out=p1[:, 0:gd], lhsT=wpairv,
                                     rhs=rhs, start=True, stop=False,
                                     perf_mode=DR)
                else:
                    nc.tensor.matmul(out=p1[:, 0:gd], lhsT=wpairv[:, 0, :],
                                     rhs=pair_t[:, loc:loc + gd],
                                     start=True, stop=False)
                    nc.tensor.matmul(
                        out=p1[:, 0:gd], lhsT=wpairv[:, 1, :],
                        rhs=pair_t[:, GROUP_COLS + loc:GROUP_COLS + loc + gd],
                        start=False, stop=False)
                nc.tensor.matmul(out=p1[:, 0:gd], lhsT=wbea16[:],
                                 rhs=bond_t[:, 0:gd], start=False, stop=True)
                h1 = ep.tile([128, 512], HDT, tag="h1")
                nc.scalar.activation(h1[:, 0:gd], p1[:, 0:gd], AF.Relu, bias=B1P)
                return h1

            def emit_tail(ci, h1):
                d, g, cols, sb, nb = p.chunks[ci]
                gd = g * d
                nsl = slice(nb, nb + g)
                p2 = epp.tile([128, 512], F32, tag="mm")
                nc.tensor.matmul(out=p2[:, 0:gd], lhsT=w2one,
                                 rhs=h1[:, 0:gd], start=True, stop=True)

                if d == 1:
                    with nc.allow_low_precision(reason="s16 evac"):
                        nc.vector.tensor_copy(s16_agg[:, nsl], p2[:, 0:g])
                    return

                # segmented sum via k-paired DoubleRow (fp8) or one
                # broadcast-output subtiled matmul (bf16)
                sq = sqp.tile([128, 512], F32, tag="sq")
                h1v = h1[:, 0:gd].rearrange("p (j k) -> p k j", k=d)
                if FP8_H1:
                    np2 = d // 2
                    for k2 in range(np2):
                        nc.tensor.matmul(
                            out=sq[:, 0:g], lhsT=w2pairv,
                            rhs=h1v[:, 2 * k2:2 * k2 + 2, :],
                            start=(k2 == 0), stop=(k2 == np2 - 1 and d % 2 == 0),
                            perf_mode=DR)
                    if d % 2 == 1:
                        nc.tensor.matmul(out=sq[:, 0:g], lhsT=w2one,
                                         rhs=h1v[:, d - 1, :],
                                         start=(np2 == 0), stop=True)
                else:
                    nc.tensor.matmul(
                        out=sq[:, None, 0:g].to_broadcast((128, d, g)),
                        lhsT=w2one, rhs=h1v,
                        start=True, stop=True, skip_group_check=True)

                p16 = ep.tile([128, 512], BF16, tag="p16")
                with nc.allow_low_precision(reason="p16 evac"):
                    nc.vector.tensor_copy(p16[:, 0:gd], p2[:, 0:gd])
                msq = ep.tile([128, 512], BF16, tag="msq")
                nc.vector.tensor_tensor(msq[:, 0:gd], p16[:, 0:gd],
                                        p16[:, 0:gd], OP.mult)
                nc.tensor.matmul(
                    out=sq[:, None, 256:256 + g].to_broadcast((128, d, g)),
                    lhsT=ident16[:],
                    rhs=msq[:, 0:gd].rearrange("p (j k) -> p k j", k=d),
                    start=True, stop=True, skip_group_check=True)

                p16v = p16[:, 0:gd].rearrange("p (j k) -> p j k", k=d)
                nc.vector.tensor_reduce(out=mn16_agg[:, nsl], in_=p16v,
                                        axis=AX.X, op=OP.min)
                nc.vector.tensor_reduce(out=mx16_agg[:, nsl], in_=p16v,
                                        axis=AX.X, op=OP.max)

                inv_d = 1.0 / d
                nc.scalar.activation(s16_agg[:, nsl], sq[:, 0:g], AF.Copy,
                                     scale=inv_d)
                nc.vector.tensor_scalar(q16_agg[:, nsl], sq[:, 256:256 + g],
                                        inv_d, None, OP.mult)

            def emit_block(t):
                nb = t * 512
                nsl = slice(nb, nb + 512)
                pieces = block_pieces[t]

                x_fm = xp.tile([128, 512], BF16, tag="x_fm")
                nc.sync.dma_start(x_fm[:], xlt_d[:, nsl])

                # batched std chain over this block's d>=2 range
                a, b = max(slo, nb), min(shi, nb + 512)
                if b > a:
                    rel = slice(a - nb, b - nb)
                    n = b - a
                    m2 = pb.tile([128, 512], BF16, tag="m2")
                    nc.gpsimd.tensor_tensor(m2[:, 0:n], s16_agg[:, a:b],
                                            s16_agg[:, a:b], OP.mult)
                    v = pb.tile([128, 512], BF16, tag="v")
                    nc.gpsimd.tensor_tensor(v[:, 0:n], q16_agg[:, a:b],
                                            m2[:, 0:n], OP.subtract)
                    vr = pb.tile([128, 512], BF16, tag="vr")
                    nc.vector.tensor_scalar(vr[:, 0:n], v[:, 0:n], 0.0, None,
                                            OP.max)
                    nc.scalar.activation(std16_agg[:, a:b], vr[:, 0:n],
                                         AF.Sqrt, bias=epsc)

                op = opp.tile([128, 512], F32, tag="op")
                mms = [dict(out=op[:], lhsT=WB(p.widx["W1X"]), rhs=x_fm[:])]
                for (d, a2, b2_) in pieces:
                    rel = slice(a2 - nb, b2_ - nb)
                    n = b2_ - a2
                    if d == 0:
                        mms.append(dict(out=op[:, rel], lhsT=CV(0),
                                        rhs=ones512[:, 0:n]))
                    elif d == 1:
                        wi = p.wd_idx[1]
                        mms.append(dict(out=op[:, rel], lhsT=WB(wi),
                                        rhs=s16_agg[:, a2:b2_]))
                        mms.append(dict(out=op[:, rel], lhsT=CV(1),
                                        rhs=ones512[:, 0:n]))
                    else:
                        wi = p.wd_idx[d]
                        mms.append(dict(out=op[:, rel], lhsT=WB(wi),
                                        rhs=s16_agg[:, a2:b2_]))
                        mms.append(dict(out=op[:, rel], lhsT=WB(wi + 1),
                                        rhs=mn16_agg[:, a2:b2_]))
                        mms.append(dict(out=op[:, rel], lhsT=WB(wi + 2),
                                        rhs=mx16_agg[:, a2:b2_]))
                        mms.append(dict(out=op[:, rel], lhsT=WB(wi + 3),
                                        rhs=std16_agg[:, a2:b2_]))
                        mms.append(dict(out=op[:, rel], lhsT=CV(d),
                                        rhs=ones512[:, 0:n]))
                for i, kw in enumerate(mms):
                    nc.tensor.matmul(start=(i == 0), stop=(i == len(mms) - 1),
                                     skip_group_check=True, **kw)

                h1p = pb.tile([128, 512], BF16, tag="h1p")
                nc.vector.tensor_scalar(h1p[:], op[:], BP1, 0.0, OP.add, OP.max)
                pp2 = mpp.tile([128, 512], F32, tag="mp")
                nc.tensor.matmul(out=pp2[:], lhsT=WB(p.widx["W2P"]), rhs=h1p[:],
                                 start=True, stop=True)
                z2 = pb.tile([128, 512], BF16, tag="z2")
                nc.scalar.activation(z2[:], pp2[:], AF.Copy)
                plin = mpp.tile([128, 512], F32, tag="mp")
                nc.tensor.matmul(out=plin[:], lhsT=WB(p.widx["WLINC"]), rhs=z2[:],
                                 start=True, stop=True)
                zf16 = pb.tile([128, 512], BF16, tag="zf16")
                nc.scalar.activation(zf16[:], plin[:], AF.Identity, bias=BLINC2)
                sq16 = pb.tile([128, 512], BF16, tag="sq16")
                nc.gpsimd.tensor_tensor(sq16[:], zf16[:], zf16[:], OP.mult)
                vs = mpp.tile([128, 512], F32, tag="mp")
                nc.tensor.matmul(out=vs[0:1, :], lhsT=onescol[:], rhs=sq16[:],
                                 start=True, stop=True)
                sd = pb.tile([1, 512], F32, tag="sd")
                nc.scalar.activation(sd[:], vs[0:1, :], AF.Sqrt,
                                     scale=1.0 / 128.0, bias=epsc[0:1, :])
                ri32 = pb.tile([1, 512], F32, tag="ri32")
                nc.vector.reciprocal_approx_fast(ri32[:], sd[:])
                ri = pb.tile([1, 512], BF16, tag="ri")
                with nc.allow_low_precision(reason="rstd bf16 broadcast"):
                    nc.vector.tensor_copy(ri[:], ri32[:])
                rb = mpp.tile([128, 512], F32, tag="mp")
                nc.tensor.matmul(out=rb[:], lhsT=ONESROW, rhs=ri[:],
                                 start=True, stop=True)
                y16 = pb.tile([128, 512], BF16, tag="y16")
                nc.vector.tensor_tensor(y16[:], zf16[:], rb[:], OP.mult)
                outbf = pb.tile([128, 512], BF16, tag="outbf")
                nc.vector.scalar_tensor_tensor(outbf[:], y16[:], 0.0, x_fm[:],
                                               OP.max, OP.add)
                nc.sync.dma_start(out_d[:, nsl], outbf[:])

            # interleaved, software-pipelined emission: head(i+1) before tail(i)
            next_block = 0

            def emit_tail_and_blocks(item):
                nonlocal next_block
                ci, h1 = item
                emit_tail(ci, h1)
                while next_block < NB and last_chunk[next_block] <= ci:
                    emit_block(next_block)
                    next_block += 1

            DEPTH = 3
            pending = []
            for gi in range(len(p.groups)):
                if gi > 0:
                    emit_gather(gi)
                for ci in p.groups[gi][2]:
                    h1 = emit_head(gi, ci)
                    pending.append((ci, h1))
                    if len(pending) > DEPTH:
                        emit_tail_and_blocks(pending.pop(0))
            for item in pending:
                emit_tail_and_blocks(item)
            while next_block < NB:
                emit_block(next_block)
                next_block += 1
    nc.compile()
    return nc


# --------------------------------------------------------------------------
# Entry point
# --------------------------------------------------------------------------

_CACHE = {}


def _get_compiled(src, dst, n_nodes):
    key = hash((src.tobytes(), dst.tobytes(), n_nodes))
    if key not in _CACHE:
        p = make_plan(src.astype(np.int64), dst.astype(np.int64), n_nodes)
        nc = build_nc(p, n_nodes)
        _CACHE[key] = (p, nc)
    return _CACHE[key]


def unshard(p, results, n_nodes):
    out = np.zeros((n_nodes, H), dtype=np.float32)
    for c in range(p.n_cores):
        o = np.asarray(results[c]["out"]).astype(np.float32)  # [128, NL]
        lay = p.layout_nodes[c]
        real = np.nonzero(lay >= 0)[0]
        out[c * p.npc + lay[real]] = o[:, real].T
    return out


def kernel(**inputs):
    atom_x = np.asarray(inputs["atom_x"], np.float32)
    bond_x = np.asarray(inputs["bond_x"], np.float32)
    ei = np.asarray(inputs["atom_edge_index"])
    src = ei[0].astype(np.int64)
    dst = ei[1].astype(np.int64)
    n_nodes = atom_x.shape[0]

    p, nc = _get_compiled(ei[0], ei[1], n_nodes)
    W = make_weights(inputs, p)
    in_maps = [make_core_inputs(p, c, atom_x, bond_x, src, dst, W)
               for c in range(p.n_cores)]
    res = run_bass_kernel_spmd(nc, in_maps, core_ids=list(range(p.n_cores)))
    return unshard(p, res.results, n_nodes)
